# revision 12
# baseline (speedup 1.0000x reference)
"""Causal self-attention (B=4, T=2048, C=1024, H=16) on 8 trn2 NeuronCores.

Sharding: 8 cores = (batch b in 0..3) x (head-half g in 0..1). Each core
computes, for its batch b and its 8 heads: the qkv projection
(column-parallel slice of w_attn), causal attention, and a row-parallel
slice of the output projection. The two cores sharing a batch produce
partial fp16 projection outputs that the host sums (+ b_proj).

Per-core device pipeline:
  q/k projection: fp8e4 DoubleRow matmuls (x8/wqk8 pairs over kc tiles,
  2 k-tiles per PE instruction = 2x throughput); host pre-scales wq by
  32 (incl 1/sqrt(D)) and wk by 64 so fp8 operands are well-conditioned;
  the 2^14 score scale is undone for free by the exp activation's scale.
  v projection fp16. scores fp16 (kT/q from SBUF), skip-max softmax with
  exp on ScalarE, causal diagonal masking via a constant triangular mask
  multiply on DVE. PV accumulates [65, tq] per head (ones column gives
  the softmax denominator). Normalize: reciprocal_approx_fast on the
  denominator row + gpsimd partition_broadcast + DVE mul (+bv bias).
  proj fp16, fp16 output DMA, host sums the partial pairs.

Schedule: scores/PV pipelined with a 1-stage lag that crosses head-pair
boundaries; qkv(j+1) and proj(j-1) are drained as fillers between units;
the last block finishes eagerly so the final proj+DMA tail is short.
"""

import os
import numpy as np
import ml_dtypes

B, T, C, H, D = 4, 2048, 1024, 16, 64
HPC = 8          # heads per core
CL = HPC * D     # 512 local channels
P = 128
NB = 512         # tq block size / matmul moving width
NT = T // P      # 16 t tiles
NJ = T // NB     # 4 tq blocks
KC = C // P      # 8 contraction tiles
QSC, KSC = 32.0, 64.0          # host weight scales (q incl 1/8)
EXPSCALE = 1.0 / (QSC * KSC * 8.0)  # = 2^-14: q had 1/8 folded before

_CACHE = {}


def _build():
    import concourse.mybir as mybir
    import concourse.tile as tile
    from concourse import bacc

    f32 = mybir.dt.float32
    mdt = mybir.dt.float16
    f8 = mybir.dt.float8e4
    AF = mybir.ActivationFunctionType
    ALU = mybir.AluOpType
    DR = mybir.MatmulPerfMode.DoubleRow

    nc = bacc.Bacc("TRN2", target_bir_lowering=False, debug=False,
                   enable_asserts=False, num_devices=8)

    x8p = nc.dram_tensor("x8p", [NJ * P, 4096], f8, kind="ExternalInput").ap()
    x16p = nc.dram_tensor("x16p", [NJ * P, 4096], mdt,
                          kind="ExternalInput").ap()
    wqk8 = nc.dram_tensor("wqk8", [4 * P, 2048], f8,
                          kind="ExternalInput").ap()
    wv = nc.dram_tensor("wv", [P, 4096], mdt, kind="ExternalInput").ap()
    wp = nc.dram_tensor("wp", [P, 4096], mdt, kind="ExternalInput").ap()
    bqk = nc.dram_tensor("bqk", [P, 8], f32, kind="ExternalInput").ap()
    bv = nc.dram_tensor("bv", [D, 8], f32, kind="ExternalInput").ap()
    cmask = nc.dram_tensor("cmask", [P, P], mdt, kind="ExternalInput").ap()
    out = nc.dram_tensor("out", [T, C], mdt, kind="ExternalOutput").ap()

    with tile.TileContext(nc) as tc:
        with tc.tile_pool(name="const", bufs=1) as const, \
             tc.tile_pool(name="kv", bufs=1) as kv, \
             tc.tile_pool(name="qy", bufs=1) as qy, \
             tc.tile_pool(name="xp8", bufs=2) as xp8, \
             tc.tile_pool(name="xp16", bufs=2) as xp16, \
             tc.tile_pool(name="pp", bufs=6) as pp, \
             tc.tile_pool(name="os", bufs=4) as osp, \
             tc.tile_pool(name="mi", bufs=2) as mi, \
             tc.tile_pool(name="scps", bufs=2, space="PSUM") as scps, \
             tc.tile_pool(name="yps", bufs=2, space="PSUM") as ypsp, \
             tc.tile_pool(name="mmps", bufs=2, space="PSUM") as mmps:

            # ---- tiny constants first: a late bias tile stalls the PSUM
            # pool behind megabytes of weights otherwise ----
            bqk_sb = const.tile([P, 8], f32, tag="bqk", name="bqk_sb")
            nc.sync.dma_start(bqk_sb[:], bqk[:, :])
            bv_sb = const.tile([D, 8], f32, tag="bv", name="bv_sb")
            nc.sync.dma_start(bv_sb[:], bv[:, :])
            cm_sb = const.tile([P, P], mdt, tag="cm", name="cm_sb")
            nc.sync.dma_start(cm_sb[:], cmask[:, :])
            # ---- q/k path (x8 + fp8 weights) on the scalar hwdge queue,
            # v path (x16 + wv) on the sync queue: parallel DMA streams ----
            xt8_0 = xp8.tile([P, 4096], f8, tag="x8", name="x8_0")
            nc.scalar.dma_start(xt8_0[:], x8p[0:P, :])
            wqk_sb = []
            for kcp in range(4):
                t = const.tile([P, 2, 1024], f8, tag=f"wqk{kcp}",
                               name=f"wqk{kcp}")
                nc.scalar.dma_start(
                    t[:], wqk8[kcp * P:(kcp + 1) * P, :].rearrange(
                        "p (u o) -> p u o", u=2))
                wqk_sb.append(t)
            xt16_0 = xp16.tile([P, 4096], mdt, tag="x16", name="x16_0")
            nc.sync.dma_start(xt16_0[:], x16p[0:P, :])
            wv_sb = const.tile([P, KC, NB], mdt, tag="wv", name="wv_sb")
            nc.sync.dma_start(wv_sb[:],
                              wv[:, :].rearrange("p (kc o) -> p kc o", kc=KC))
            wp_sb = const.tile([P, 4, 1024], mdt, tag="wp", name="wp_sb")

            # ---- persistent attention state ----
            kT_sb = [kv.tile([P, T], mdt, tag=f"kT{i}", name=f"kT{i}")
                     for i in range(CL // P)]
            v_sb = [kv.tile([P, HPC, 65], mdt, tag=f"v{i}", name=f"v{i}")
                    for i in range(NT)]
            for i in range(NT):
                nc.vector.memset(v_sb[i][:, :, 64:65], 1.0)

            def emit_qkv(j, xt8, xt16):
                if xt8 is None:
                    xt8 = xp8.tile([P, 4096], f8, tag="x8", name="x8")
                    nc.sync.dma_start(xt8[:], x8p[j * P:(j + 1) * P, :])
                    xt16 = xp16.tile([P, 4096], mdt, tag="x16", name="x16")
                    nc.sync.dma_start(xt16[:], x16p[j * P:(j + 1) * P, :])
                    yield  # DMA-only step: prefetch before any PE work queues
                x83 = xt8[:].rearrange("p (kcp u n) -> p kcp u n", kcp=4, u=2)
                x163 = xt16[:].rearrange("p (kc n) -> p kc n", kc=KC)
                q_cur = qs[j % 3]
                for ct in (0, 4, 1, 5, 2, 6, 3, 7):
                    ps = mmps.tile([P, NB], f32, tag="mm", name="ps")
                    for kcp in range(4):
                        nc.tensor.matmul(
                            ps[:],
                            wqk_sb[kcp][:, :, ct * P:(ct + 1) * P],
                            x83[:, kcp],
                            start=(kcp == 0), stop=(kcp == 3),
                            perf_mode=DR)
                    dst = (q_cur[ct][:] if ct < 4
                           else kT_sb[ct - 4][:, j * NB:(j + 1) * NB])
                    nc.vector.tensor_scalar_add(dst, ps[:],
                                                bqk_sb[:, ct:ct + 1])
                    yield
                for tl in range(4):
                    tt = 4 * j + tl
                    ps = mmps.tile([P, NB], f32, tag="mm", name="ps")
                    for kc in range(KC):
                        nc.tensor.matmul(ps[:],
                                         x163[:, kc, tl * P:(tl + 1) * P],
                                         wv_sb[:, kc, :],
                                         start=(kc == 0), stop=(kc == KC - 1))
                    nc.vector.tensor_copy(
                        v_sb[tt][:, :, 0:64],
                        ps[:].rearrange("p (h w) -> p h w", h=HPC))
                    yield

            def drain(gens, n):
                done = 0
                while gens and done < n:
                    try:
                        next(gens[0])
                        done += 1
                    except StopIteration:
                        gens.pop(0)

            def emit_attn(j, q_cur, y_cur, fast, slow, proj, last_block):
                n_tk = 4 * (j + 1)
                # diagonal tiles first: their longer chains (exp+mask)
                # hide under the full tiles that follow
                order = list(range(4 * j, n_tk)) + list(range(0, 4 * j))

                def stage_scores(hp, tkt):
                    off = max(0, (tkt - 4 * j) * P)
                    sc = scps.tile([P, 2, NB], f32, tag="sc", name="sc")
                    nc.tensor.matmul(
                        sc[:, 0, off:NB],
                        kT_sb[hp][0:D, tkt * P:(tkt + 1) * P],
                        q_cur[hp][0:D, off:], start=True, stop=True)
                    nc.tensor.matmul(
                        sc[:, 1, off:NB],
                        kT_sb[hp][D:2 * D, tkt * P:(tkt + 1) * P],
                        q_cur[hp][D:2 * D, off:], start=True, stop=True)
                    pt = pp.tile([P, 2, NB], mdt, tag="pt", name="pt")
                    nc.scalar.activation(pt[:, :, off:], sc[:, :, off:],
                                         AF.Exp, scale=EXPSCALE)
                    if tkt >= 4 * j:
                        # split the two causal masks across engines to keep
                        # the vector queue shallow
                        win = pt[:, 0, off:off + P]
                        nc.vector.tensor_tensor(win, win, cm_sb[:], ALU.mult)
                        win = pt[:, 1, off:off + P]
                        nc.gpsimd.tensor_tensor(win, win, cm_sb[:], ALU.mult)
                    return tkt, off, pt

                def stage_pv(st):
                    hp, (tkt, off, pt), first, last, yp0, yp1 = st
                    h0, h1 = 2 * hp, 2 * hp + 1
                    nc.tensor.matmul(
                        yp0[:, off:NB], v_sb[tkt][:, h0, :], pt[:, 0, off:NB],
                        start=first, stop=last)
                    nc.tensor.matmul(
                        yp1[:, off:NB], v_sb[tkt][:, h1, :], pt[:, 1, off:NB],
                        start=first, stop=last)

                def evac(pyp0, pyp1):
                    # denominator rows to partition-0 tiles first
                    # (partition_broadcast reads partition 0), reciprocals
                    # right behind so the finish chain unblocks early;
                    # bulk y copies last
                    ysb0 = mi.tile([D, NB], f32, tag="ysb0", name="ysb0")
                    ysb1 = mi.tile([D, NB], f32, tag="ysb1", name="ysb1")
                    r0 = mi.tile([1, NB], f32, tag="r0", name="r0")
                    r1 = mi.tile([1, NB], f32, tag="r1", name="r1")
                    nc.vector.tensor_copy(r0[:], pyp0[64:65, :])
                    nc.vector.tensor_copy(r1[:], pyp1[64:65, :])
                    nc.vector.reciprocal_approx_fast(r0[:], r0[:])
                    nc.vector.reciprocal_approx_fast(r1[:], r1[:])
                    nc.vector.tensor_copy(ysb0[:], pyp0[0:D, :])
                    nc.vector.tensor_copy(ysb1[:], pyp1[0:D, :])
                    return ysb0, r0, ysb1, r1

                def finish(hp, ysb0, r0, ysb1, r1):
                    # both broadcasts issued first: gpsimd runs them
                    # back-to-back while the vector queue does the mults
                    rb0 = mi.tile([D, NB], f32, tag="rb", name="rb0")
                    rb1 = mi.tile([D, NB], f32, tag="rb", name="rb1")
                    nc.gpsimd.partition_broadcast(rb0[:], r0[:])
                    nc.gpsimd.partition_broadcast(rb1[:], r1[:])
                    for h, ysb, rb in ((2 * hp, ysb0, rb0),
                                       (2 * hp + 1, ysb1, rb1)):
                        po = D * (h % 2)
                        y_dst = y_cur[hp][po:po + D, :]
                        nc.vector.tensor_tensor(y_dst, ysb[0:D, :],
                                                rb[:], ALU.mult)
                        nc.vector.tensor_scalar_add(y_dst, y_dst,
                                                    bv_sb[:, h:h + 1])

                pend_pv = None
                pend_fin = None
                ucount = 0
                for hp in range(4):
                    yp0 = ypsp.tile([65, NB], f32, tag="yps", name="yp0")
                    yp1 = ypsp.tile([65, NB], f32, tag="yps", name="yp1")
                    for ui, tkt in enumerate(order):
                        st = stage_scores(hp, tkt)
                        prev = pend_pv
                        pend_pv = (hp, st, ui == 0, ui == n_tk - 1, yp0, yp1)
                        if prev is not None:
                            stage_pv(prev)
                            if prev[3]:      # closed out a head-pair
                                ev = evac(prev[4], prev[5])
                                drain(fast if fast else slow, 1)
                                if last_block:
                                    finish(prev[0], *ev)
                                else:
                                    if pend_fin is not None:
                                        finish(*pend_fin)
                                    pend_fin = (prev[0],) + ev
                        ucount += 1
                        # filler cadence: proj spread thin, qkv(j+1) eagerly
                        # for exp-latency cover, qkv(j+2) as slow backfill
                        if proj and ucount % 5 == 0:
                            drain(proj, 1)
                        elif fast:
                            drain(fast, 1)
                        elif slow and ucount % 3 == 0:
                            drain(slow, 1)
                # last unit's PV + its head-pair epilogue
                hp = pend_pv[0]
                stage_pv(pend_pv)
                ev = evac(pend_pv[4], pend_pv[5])
                if pend_fin is not None:
                    finish(*pend_fin)
                finish(hp, *ev)
                # qkv(j+1) must complete before attn(j+1); proj(j-1) must
                # complete before attn(j+1) overwrites its y tiles
                drain(fast, 99)
                drain(proj, 99)

            def emit_proj(j, y_cur):
                wp3 = wp_sb[:]
                for ts in range(4):
                    for nb2 in range(2):
                        pps = mmps.tile([P, NB], f32, tag="mm", name="pps")
                        for kc in range(CL // P):
                            nc.tensor.matmul(
                                pps[:],
                                y_cur[kc][:, ts * P:(ts + 1) * P],
                                wp3[:, kc, nb2 * NB:(nb2 + 1) * NB],
                                start=(kc == 0), stop=(kc == CL // P - 1))
                        ost = osp.tile([P, NB], mdt, tag="ost", name="ost")
                        nc.vector.tensor_copy(ost[:], pps[:])
                        nc.sync.dma_start(
                            out[j * NB + ts * P:j * NB + (ts + 1) * P,
                                nb2 * NB:(nb2 + 1) * NB], ost[:])
                        yield

            qs = [[qy.tile([P, NB], mdt, tag=f"q{i}{s}", name=f"q{i}{s}")
                   for i in range(CL // P)] for s in ("a", "b", "c")]
            ys = [[qy.tile([P, NB], mdt, tag=f"y{i}{s}", name=f"y{i}{s}")
                   for i in range(CL // P)] for s in ("a", "b")]
            for _ in emit_qkv(0, xt8_0, xt16_0):
                pass
            # proj weights: needed first at ~attn(1); queued after x/qk/v
            nc.sync.dma_start(wp_sb[:],
                              wp[:, :].rearrange("p (kc o) -> p kc o", kc=4))
            qkv_g = {jj: emit_qkv(jj, None, None) for jj in range(1, NJ)}
            proj = []
            for j in range(NJ):
                fast = [qkv_g[j + 1]] if j + 1 < NJ else []
                slow = [qkv_g[j + 2]] if j + 2 < NJ else []
                emit_attn(j, qs[j % 3], ys[j % 2], fast, slow, proj,
                          last_block=(j == NJ - 1))
                if j + 1 < NJ:
                    proj = [emit_proj(j, ys[j % 2])]
            for _ in emit_proj(NJ - 1, ys[(NJ - 1) % 2]):
                pass
    nc.compile()
    return nc


def _prep_in_maps(x, w_attn, b_attn, w_proj):
    x = np.asarray(x, np.float32)
    w_attn = np.asarray(w_attn, np.float32)
    b_attn = np.asarray(b_attn, np.float32)
    w_proj = np.asarray(w_proj, np.float32)
    f16 = np.float16
    f8 = ml_dtypes.float8_e4m3
    cmask = np.triu(np.ones((P, P), np.float32)).astype(f16)
    in_maps = []
    for core in range(8):
        b, g = divmod(core, 2)
        hs = slice(g * CL, (g + 1) * CL)
        wq = w_attn[:, 0:C][:, hs] * QSC   # includes the 1/8 score scale
        wk = w_attn[:, C:2 * C][:, hs] * KSC
        wvv = w_attn[:, 2 * C:3 * C][:, hs]
        bq = b_attn[0:C][hs] * QSC
        bk = b_attn[C:2 * C][hs] * KSC
        bvv = b_attn[2 * C:3 * C][hs]
        xT = np.ascontiguousarray(x[b].T)            # [C, T]
        # x16p[j*128+p, (kc, n)] = xT[kc*128+p, j*512+n]
        x16p = xT.reshape(KC, P, NJ, NB).transpose(2, 1, 0, 3)
        x16p = np.ascontiguousarray(x16p.reshape(NJ * P, KC * NB)).astype(f16)
        # x8p[j*128+p, (kcp, u, n)] = xT[(2kcp+u)*128+p, j*512+n]
        x8p = xT.reshape(4, 2, P, NJ, NB).transpose(3, 2, 0, 1, 4)
        x8p = np.ascontiguousarray(x8p.reshape(NJ * P, 4096)).astype(f8)
        # wqk8[kcp*128+p, (u, o)] = [wq|wk][(2kcp+u)*128+p, o]
        wqk = np.concatenate([wq, wk], axis=1)       # [C, 1024]
        wqk8 = wqk.reshape(4, 2, P, 1024).transpose(0, 2, 1, 3)
        wqk8 = np.ascontiguousarray(wqk8.reshape(4 * P, 2048)).astype(f8)
        # wv[p, (kc, o)] = wvv[kc*128+p, o]
        wv16 = wvv.reshape(KC, P, CL).transpose(1, 0, 2)
        wv16 = np.ascontiguousarray(wv16.reshape(P, KC * CL)).astype(f16)
        # wp[p, (kc, o)] = w_proj[hs][kc*128+p, o]
        wpc = w_proj[hs, :].reshape(4, P, C).transpose(1, 0, 2)
        wpc = np.ascontiguousarray(wpc.reshape(P, 4 * C)).astype(f16)
        in_maps.append({
            "x8p": x8p,
            "x16p": x16p,
            "wqk8": wqk8,
            "wv": wv16,
            "wp": wpc,
            "bqk": np.ascontiguousarray(
                np.concatenate([bq, bk]).reshape(8, P).T),
            "bv": np.ascontiguousarray(bvv.reshape(8, D).T),
            "cmask": cmask,
        })
    return in_maps


def _install_ntff_hook():
    """The image lacks antenv.axon_hooks; recreate it so
    run_bass_kernel_spmd(trace=True) can capture NTFF profiles."""
    import sys
    import types
    try:
        from antenv.axon_hooks import get_axon_ntff_profile_hook  # noqa: F401
        return
    except ImportError:
        pass
    import importlib.util
    spec = importlib.util.spec_from_file_location(
        "_trn_boot", "/root/.axon_site/trn_agent_boot/trn_boot.py")
    if spec is None or not os.path.exists("/opt/axon/libaxon_pjrt.so"):
        return
    boot = importlib.util.module_from_spec(spec)
    try:
        spec.loader.exec_module(boot)
        hook = boot._ntff_profile_via_ctypes("/opt/axon/libaxon_pjrt.so")
    except Exception:
        return
    mod = types.ModuleType("antenv.axon_hooks")
    mod.get_axon_ntff_profile_hook = lambda: hook
    mod.set_axon_ntff_profile_hook = lambda h: None
    sys.modules["antenv.axon_hooks"] = mod


def _run(in_maps, trace=False, tmpdir=None):
    from concourse import bass_utils
    if trace:
        _install_ntff_hook()
        bass_utils.upload_artifacts = lambda d: "local://" + str(d)
    if "nc" not in _CACHE:
        _CACHE["nc"] = _build()
    return bass_utils.run_bass_kernel_spmd(
        _CACHE["nc"], in_maps, core_ids=list(range(8)),
        trace=trace, tmpdir=tmpdir)


def kernel(x, w_attn, b_attn, w_proj, b_proj):
    in_maps = _prep_in_maps(x, w_attn, b_attn, w_proj)
    res = _run(in_maps, trace=bool(int(os.environ.get("KERNEL_TRACE", "0"))))
    b_proj = np.asarray(b_proj, np.float32)
    out = np.zeros((B, T, C), np.float32)
    for core in range(8):
        out[core // 2] += res.results[core]["out"].astype(np.float32)
    out += b_proj[None, None, :]
    return out


# revision 13
# speedup vs baseline: 1.9157x; 1.9157x over previous
"""Causal self-attention (B=4, T=2048, C=1024, H=16) on 8 trn2 NeuronCores.

Sharding: 8 cores = (batch b in 0..3) x (head-half g in 0..1). Each core
computes, for its batch b and its 8 heads: the qkv projection
(column-parallel slice of w_attn), causal attention, and a row-parallel
slice of the output projection. The two cores sharing a batch produce
partial fp16 projection outputs that the host sums (+ b_proj).

Per-core device pipeline:
  q/k projection: fp8e4 DoubleRow matmuls (x8/wqk8 pairs over kc tiles,
  2 k-tiles per PE instruction = 2x throughput); host pre-scales wq by
  32 (incl 1/sqrt(D)) and wk by 64 so fp8 operands are well-conditioned;
  the 2^14 score scale is undone for free by the exp activation's scale.
  v projection fp16. scores fp16 (kT/q from SBUF), skip-max softmax with
  exp on ScalarE, causal diagonal masking via a constant triangular mask
  multiply on DVE. PV accumulates [65, tq] per head (ones column gives
  the softmax denominator). Normalize: reciprocal_approx_fast on the
  denominator row + gpsimd partition_broadcast + DVE mul (+bv bias).
  proj fp16, fp16 output DMA, host sums the partial pairs.

Schedule: scores/PV pipelined with a 1-stage lag that crosses head-pair
boundaries; qkv(j+1) and proj(j-1) are drained as fillers between units;
the last block finishes eagerly so the final proj+DMA tail is short.
"""

import os
import numpy as np
import ml_dtypes

B, T, C, H, D = 4, 2048, 1024, 16, 64
HPC = 8          # heads per core
CL = HPC * D     # 512 local channels
P = 128
NB = 512         # tq block size / matmul moving width
NT = T // P      # 16 t tiles
NJ = T // NB     # 4 tq blocks
KC = C // P      # 8 contraction tiles
QSC, KSC = 32.0, 64.0          # host weight scales (q incl 1/8)
EXPSCALE = 1.0 / (QSC * KSC * 8.0)  # = 2^-14: q had 1/8 folded before

_CACHE = {}


def _build():
    import concourse.mybir as mybir
    import concourse.tile as tile
    from concourse import bacc

    f32 = mybir.dt.float32
    mdt = mybir.dt.float16
    f8 = mybir.dt.float8e4
    AF = mybir.ActivationFunctionType
    ALU = mybir.AluOpType
    DR = mybir.MatmulPerfMode.DoubleRow

    nc = bacc.Bacc("TRN2", target_bir_lowering=False, debug=False,
                   enable_asserts=False, num_devices=8)

    x8p = nc.dram_tensor("x8p", [NJ * P, 4096], f8, kind="ExternalInput").ap()
    x16p = nc.dram_tensor("x16p", [NJ * P, 4096], mdt,
                          kind="ExternalInput").ap()
    wqk8 = nc.dram_tensor("wqk8", [4 * P, 2048], f8,
                          kind="ExternalInput").ap()
    wv = nc.dram_tensor("wv", [P, 4096], mdt, kind="ExternalInput").ap()
    wp = nc.dram_tensor("wp", [P, 4096], mdt, kind="ExternalInput").ap()
    bqk = nc.dram_tensor("bqk", [P, 8], f32, kind="ExternalInput").ap()
    bv = nc.dram_tensor("bv", [D, 8], f32, kind="ExternalInput").ap()
    cmask = nc.dram_tensor("cmask", [P, P], mdt, kind="ExternalInput").ap()
    out = nc.dram_tensor("out", [T, C], mdt, kind="ExternalOutput").ap()

    with tile.TileContext(nc) as tc:
        with tc.tile_pool(name="const", bufs=1) as const, \
             tc.tile_pool(name="kv", bufs=1) as kv, \
             tc.tile_pool(name="qy", bufs=1) as qy, \
             tc.tile_pool(name="xp8", bufs=2) as xp8, \
             tc.tile_pool(name="xp16", bufs=2) as xp16, \
             tc.tile_pool(name="pp", bufs=6) as pp, \
             tc.tile_pool(name="os", bufs=4) as osp, \
             tc.tile_pool(name="mi", bufs=2) as mi, \
             tc.tile_pool(name="scps", bufs=2, space="PSUM") as scps, \
             tc.tile_pool(name="yps", bufs=2, space="PSUM") as ypsp, \
             tc.tile_pool(name="mmps", bufs=2, space="PSUM") as mmps:

            # ---- tiny constants first: a late bias tile stalls the PSUM
            # pool behind megabytes of weights otherwise ----
            bqk_sb = const.tile([P, 8], f32, tag="bqk", name="bqk_sb")
            nc.sync.dma_start(bqk_sb[:], bqk[:, :])
            bv_sb = const.tile([D, 8], f32, tag="bv", name="bv_sb")
            nc.sync.dma_start(bv_sb[:], bv[:, :])
            cm_sb = const.tile([P, P], mdt, tag="cm", name="cm_sb")
            nc.sync.dma_start(cm_sb[:], cmask[:, :])
            # ---- q/k path (x8 + fp8 weights) on the scalar hwdge queue,
            # v path (x16 + wv) on the sync queue: parallel DMA streams ----
            xt8_0 = xp8.tile([P, 4096], f8, tag="x8", name="x8_0")
            nc.scalar.dma_start(xt8_0[:], x8p[0:P, :])
            wqk_sb = []
            for kcp in range(4):
                t = const.tile([P, 2, 1024], f8, tag=f"wqk{kcp}",
                               name=f"wqk{kcp}")
                nc.scalar.dma_start(
                    t[:], wqk8[kcp * P:(kcp + 1) * P, :].rearrange(
                        "p (u o) -> p u o", u=2))
                wqk_sb.append(t)
            xt16_0 = xp16.tile([P, 4096], mdt, tag="x16", name="x16_0")
            nc.sync.dma_start(xt16_0[:], x16p[0:P, :])
            wv_sb = const.tile([P, KC, NB], mdt, tag="wv", name="wv_sb")
            nc.sync.dma_start(wv_sb[:],
                              wv[:, :].rearrange("p (kc o) -> p kc o", kc=KC))
            wp_sb = const.tile([P, 4, 1024], mdt, tag="wp", name="wp_sb")

            # ---- persistent attention state ----
            kT_sb = [kv.tile([P, T], mdt, tag=f"kT{i}", name=f"kT{i}")
                     for i in range(CL // P)]
            v_sb = [kv.tile([P, HPC, 65], mdt, tag=f"v{i}", name=f"v{i}")
                    for i in range(NT)]
            for i in range(NT):
                nc.vector.memset(v_sb[i][:, :, 64:65], 1.0)

            def emit_qkv(j, xt8, xt16):
                if xt8 is None:
                    xt8 = xp8.tile([P, 4096], f8, tag="x8", name="x8")
                    nc.sync.dma_start(xt8[:], x8p[j * P:(j + 1) * P, :])
                    xt16 = xp16.tile([P, 4096], mdt, tag="x16", name="x16")
                    nc.sync.dma_start(xt16[:], x16p[j * P:(j + 1) * P, :])
                    yield  # DMA-only step: prefetch before any PE work queues
                x83 = xt8[:].rearrange("p (kcp u n) -> p kcp u n", kcp=4, u=2)
                x163 = xt16[:].rearrange("p (kc n) -> p kc n", kc=KC)
                q_cur = qs[j % 3]
                for ct in (0, 4, 1, 5, 2, 6, 3, 7):
                    ps = mmps.tile([P, NB], f32, tag="mm", name="ps")
                    for kcp in range(4):
                        nc.tensor.matmul(
                            ps[:],
                            wqk_sb[kcp][:, :, ct * P:(ct + 1) * P],
                            x83[:, kcp],
                            start=(kcp == 0), stop=(kcp == 3),
                            perf_mode=DR)
                    dst = (q_cur[ct][:] if ct < 4
                           else kT_sb[ct - 4][:, j * NB:(j + 1) * NB])
                    nc.vector.tensor_scalar_add(dst, ps[:],
                                                bqk_sb[:, ct:ct + 1])
                    yield
                for tl in range(4):
                    tt = 4 * j + tl
                    ps = mmps.tile([P, NB], f32, tag="mm", name="ps")
                    for kc in range(KC):
                        nc.tensor.matmul(ps[:],
                                         x163[:, kc, tl * P:(tl + 1) * P],
                                         wv_sb[:, kc, :],
                                         start=(kc == 0), stop=(kc == KC - 1))
                    nc.vector.tensor_copy(
                        v_sb[tt][:, :, 0:64],
                        ps[:].rearrange("p (h w) -> p h w", h=HPC))
                    yield

            def drain(gens, n):
                done = 0
                while gens and done < n:
                    try:
                        next(gens[0])
                        done += 1
                    except StopIteration:
                        gens.pop(0)

            def emit_attn(j, q_cur, y_cur, fast, slow, proj, last_block):
                n_tk = 4 * (j + 1)
                # diagonal tiles first: their longer chains (exp+mask)
                # hide under the full tiles that follow
                order = list(range(4 * j, n_tk)) + list(range(0, 4 * j))

                def stage_scores(hp, tkt):
                    off = max(0, (tkt - 4 * j) * P)
                    sc = scps.tile([P, 2, NB], f32, tag="sc", name="sc")
                    nc.tensor.matmul(
                        sc[:, 0, off:NB],
                        kT_sb[hp][0:D, tkt * P:(tkt + 1) * P],
                        q_cur[hp][0:D, off:], start=True, stop=True)
                    nc.tensor.matmul(
                        sc[:, 1, off:NB],
                        kT_sb[hp][D:2 * D, tkt * P:(tkt + 1) * P],
                        q_cur[hp][D:2 * D, off:], start=True, stop=True)
                    pt = pp.tile([P, 2, NB], mdt, tag="pt", name="pt")
                    nc.scalar.activation(pt[:, :, off:], sc[:, :, off:],
                                         AF.Exp, scale=EXPSCALE)
                    if tkt >= 4 * j:
                        # both masks on vector: gpsimd's in-order queue must
                        # stay shallow for the partition_broadcasts (sharing
                        # it creates a cross-queue convoy)
                        for u in range(2):
                            win = pt[:, u, off:off + P]
                            nc.vector.tensor_tensor(win, win, cm_sb[:],
                                                    ALU.mult)
                    return tkt, off, pt

                def stage_pv(st):
                    hp, (tkt, off, pt), first, last, yp0, yp1 = st
                    h0, h1 = 2 * hp, 2 * hp + 1
                    nc.tensor.matmul(
                        yp0[:, off:NB], v_sb[tkt][:, h0, :], pt[:, 0, off:NB],
                        start=first, stop=last)
                    nc.tensor.matmul(
                        yp1[:, off:NB], v_sb[tkt][:, h1, :], pt[:, 1, off:NB],
                        start=first, stop=last)

                def evac(pyp0, pyp1):
                    # denominator rows to partition-0 tiles first
                    # (partition_broadcast reads partition 0), reciprocals
                    # right behind so the finish chain unblocks early;
                    # bulk y copies last
                    ysb0 = mi.tile([D, NB], f32, tag="ysb0", name="ysb0")
                    ysb1 = mi.tile([D, NB], f32, tag="ysb1", name="ysb1")
                    r0 = mi.tile([1, NB], f32, tag="r0", name="r0")
                    r1 = mi.tile([1, NB], f32, tag="r1", name="r1")
                    nc.vector.tensor_copy(r0[:], pyp0[64:65, :])
                    nc.vector.tensor_copy(r1[:], pyp1[64:65, :])
                    nc.vector.reciprocal_approx_fast(r0[:], r0[:])
                    nc.vector.reciprocal_approx_fast(r1[:], r1[:])
                    nc.vector.tensor_copy(ysb0[:], pyp0[0:D, :])
                    nc.vector.tensor_copy(ysb1[:], pyp1[0:D, :])
                    return ysb0, r0, ysb1, r1

                def finish(hp, ysb0, r0, ysb1, r1):
                    # both broadcasts issued first: gpsimd runs them
                    # back-to-back while the vector queue does the mults
                    rb0 = mi.tile([D, NB], f32, tag="rb", name="rb0")
                    rb1 = mi.tile([D, NB], f32, tag="rb", name="rb1")
                    nc.gpsimd.partition_broadcast(rb0[:], r0[:])
                    nc.gpsimd.partition_broadcast(rb1[:], r1[:])
                    for h, ysb, rb in ((2 * hp, ysb0, rb0),
                                       (2 * hp + 1, ysb1, rb1)):
                        po = D * (h % 2)
                        y_dst = y_cur[hp][po:po + D, :]
                        nc.vector.tensor_tensor(y_dst, ysb[0:D, :],
                                                rb[:], ALU.mult)
                        nc.vector.tensor_scalar_add(y_dst, y_dst,
                                                    bv_sb[:, h:h + 1])

                pend_pv = None
                pend_fin = None
                ucount = 0
                for hp in range(4):
                    yp0 = ypsp.tile([65, NB], f32, tag="yps", name="yp0")
                    yp1 = ypsp.tile([65, NB], f32, tag="yps", name="yp1")
                    for ui, tkt in enumerate(order):
                        st = stage_scores(hp, tkt)
                        prev = pend_pv
                        pend_pv = (hp, st, ui == 0, ui == n_tk - 1, yp0, yp1)
                        if prev is not None:
                            stage_pv(prev)
                            if prev[3]:      # closed out a head-pair
                                ev = evac(prev[4], prev[5])
                                drain(fast if fast else slow, 1)
                                if last_block:
                                    finish(prev[0], *ev)
                                else:
                                    if pend_fin is not None:
                                        finish(*pend_fin)
                                    pend_fin = (prev[0],) + ev
                        ucount += 1
                        # filler cadence: proj spread thin, qkv(j+1) eagerly
                        # for exp-latency cover, qkv(j+2) as slow backfill
                        if proj and ucount % 5 == 0:
                            drain(proj, 1)
                        elif fast:
                            drain(fast, 1)
                        elif slow and ucount % 3 == 0:
                            drain(slow, 1)
                # last unit's PV + its head-pair epilogue
                hp = pend_pv[0]
                stage_pv(pend_pv)
                ev = evac(pend_pv[4], pend_pv[5])
                if pend_fin is not None:
                    finish(*pend_fin)
                finish(hp, *ev)
                # qkv(j+1) must complete before attn(j+1); proj(j-1) must
                # complete before attn(j+1) overwrites its y tiles
                drain(fast, 99)
                drain(proj, 99)

            def emit_proj(j, y_cur):
                wp3 = wp_sb[:]
                for ts in range(4):
                    for nb2 in range(2):
                        pps = mmps.tile([P, NB], f32, tag="mm", name="pps")
                        for kc in range(CL // P):
                            nc.tensor.matmul(
                                pps[:],
                                y_cur[kc][:, ts * P:(ts + 1) * P],
                                wp3[:, kc, nb2 * NB:(nb2 + 1) * NB],
                                start=(kc == 0), stop=(kc == CL // P - 1))
                        ost = osp.tile([P, NB], mdt, tag="ost", name="ost")
                        nc.vector.tensor_copy(ost[:], pps[:])
                        nc.sync.dma_start(
                            out[j * NB + ts * P:j * NB + (ts + 1) * P,
                                nb2 * NB:(nb2 + 1) * NB], ost[:])
                        yield

            qs = [[qy.tile([P, NB], mdt, tag=f"q{i}{s}", name=f"q{i}{s}")
                   for i in range(CL // P)] for s in ("a", "b", "c")]
            ys = [[qy.tile([P, NB], mdt, tag=f"y{i}{s}", name=f"y{i}{s}")
                   for i in range(CL // P)] for s in ("a", "b")]
            for _ in emit_qkv(0, xt8_0, xt16_0):
                pass
            # proj weights: needed first at ~attn(1); queued after x/qk/v
            nc.sync.dma_start(wp_sb[:],
                              wp[:, :].rearrange("p (kc o) -> p kc o", kc=4))
            qkv_g = {jj: emit_qkv(jj, None, None) for jj in range(1, NJ)}
            proj = []
            for j in range(NJ):
                fast = [qkv_g[j + 1]] if j + 1 < NJ else []
                slow = [qkv_g[j + 2]] if j + 2 < NJ else []
                emit_attn(j, qs[j % 3], ys[j % 2], fast, slow, proj,
                          last_block=(j == NJ - 1))
                if j + 1 < NJ:
                    proj = [emit_proj(j, ys[j % 2])]
            for _ in emit_proj(NJ - 1, ys[(NJ - 1) % 2]):
                pass
    nc.compile()
    return nc


def _prep_in_maps(x, w_attn, b_attn, w_proj):
    x = np.asarray(x, np.float32)
    w_attn = np.asarray(w_attn, np.float32)
    b_attn = np.asarray(b_attn, np.float32)
    w_proj = np.asarray(w_proj, np.float32)
    f16 = np.float16
    f8 = ml_dtypes.float8_e4m3
    cmask = np.triu(np.ones((P, P), np.float32)).astype(f16)
    in_maps = []
    for core in range(8):
        b, g = divmod(core, 2)
        hs = slice(g * CL, (g + 1) * CL)
        wq = w_attn[:, 0:C][:, hs] * QSC   # includes the 1/8 score scale
        wk = w_attn[:, C:2 * C][:, hs] * KSC
        wvv = w_attn[:, 2 * C:3 * C][:, hs]
        bq = b_attn[0:C][hs] * QSC
        bk = b_attn[C:2 * C][hs] * KSC
        bvv = b_attn[2 * C:3 * C][hs]
        xT = np.ascontiguousarray(x[b].T)            # [C, T]
        # x16p[j*128+p, (kc, n)] = xT[kc*128+p, j*512+n]
        x16p = xT.reshape(KC, P, NJ, NB).transpose(2, 1, 0, 3)
        x16p = np.ascontiguousarray(x16p.reshape(NJ * P, KC * NB)).astype(f16)
        # x8p[j*128+p, (kcp, u, n)] = xT[(2kcp+u)*128+p, j*512+n]
        x8p = xT.reshape(4, 2, P, NJ, NB).transpose(3, 2, 0, 1, 4)
        x8p = np.ascontiguousarray(x8p.reshape(NJ * P, 4096)).astype(f8)
        # wqk8[kcp*128+p, (u, o)] = [wq|wk][(2kcp+u)*128+p, o]
        wqk = np.concatenate([wq, wk], axis=1)       # [C, 1024]
        wqk8 = wqk.reshape(4, 2, P, 1024).transpose(0, 2, 1, 3)
        wqk8 = np.ascontiguousarray(wqk8.reshape(4 * P, 2048)).astype(f8)
        # wv[p, (kc, o)] = wvv[kc*128+p, o]
        wv16 = wvv.reshape(KC, P, CL).transpose(1, 0, 2)
        wv16 = np.ascontiguousarray(wv16.reshape(P, KC * CL)).astype(f16)
        # wp[p, (kc, o)] = w_proj[hs][kc*128+p, o]
        wpc = w_proj[hs, :].reshape(4, P, C).transpose(1, 0, 2)
        wpc = np.ascontiguousarray(wpc.reshape(P, 4 * C)).astype(f16)
        in_maps.append({
            "x8p": x8p,
            "x16p": x16p,
            "wqk8": wqk8,
            "wv": wv16,
            "wp": wpc,
            "bqk": np.ascontiguousarray(
                np.concatenate([bq, bk]).reshape(8, P).T),
            "bv": np.ascontiguousarray(bvv.reshape(8, D).T),
            "cmask": cmask,
        })
    return in_maps


def _install_ntff_hook():
    """The image lacks antenv.axon_hooks; recreate it so
    run_bass_kernel_spmd(trace=True) can capture NTFF profiles."""
    import sys
    import types
    try:
        from antenv.axon_hooks import get_axon_ntff_profile_hook  # noqa: F401
        return
    except ImportError:
        pass
    import importlib.util
    spec = importlib.util.spec_from_file_location(
        "_trn_boot", "/root/.axon_site/trn_agent_boot/trn_boot.py")
    if spec is None or not os.path.exists("/opt/axon/libaxon_pjrt.so"):
        return
    boot = importlib.util.module_from_spec(spec)
    try:
        spec.loader.exec_module(boot)
        hook = boot._ntff_profile_via_ctypes("/opt/axon/libaxon_pjrt.so")
    except Exception:
        return
    mod = types.ModuleType("antenv.axon_hooks")
    mod.get_axon_ntff_profile_hook = lambda: hook
    mod.set_axon_ntff_profile_hook = lambda h: None
    sys.modules["antenv.axon_hooks"] = mod


def _run(in_maps, trace=False, tmpdir=None):
    from concourse import bass_utils
    if trace:
        _install_ntff_hook()
        bass_utils.upload_artifacts = lambda d: "local://" + str(d)
    if "nc" not in _CACHE:
        _CACHE["nc"] = _build()
    return bass_utils.run_bass_kernel_spmd(
        _CACHE["nc"], in_maps, core_ids=list(range(8)),
        trace=trace, tmpdir=tmpdir)


def kernel(x, w_attn, b_attn, w_proj, b_proj):
    in_maps = _prep_in_maps(x, w_attn, b_attn, w_proj)
    res = _run(in_maps, trace=bool(int(os.environ.get("KERNEL_TRACE", "0"))))
    b_proj = np.asarray(b_proj, np.float32)
    out = np.zeros((B, T, C), np.float32)
    for core in range(8):
        out[core // 2] += res.results[core]["out"].astype(np.float32)
    out += b_proj[None, None, :]
    return out
